# revision 19
# baseline (speedup 1.0000x reference)
"""Trainium2 Bass kernel for causal multi-head attention (b=2, n=2048, d=1024, h=16).

Sharding: 8 cores; core c handles batch (c // 4) and the 4 heads
[4*(c%4), 4*(c%4)+4).  Each core computes its heads' attention plus its
partial output projection y_part = O_heads @ Wo[:, cols].T ; the host sums
the 4 per-batch partials and adds bo.

On-device pipeline per core:
  xT (d-major) -> QT,KT [hd, n] (f32r) and V [n, hd] (bf16) projections
  ST tile [k,q] = KT-chunk.T x QTz_h      (f32r, K=128: both heads' KT rows
                                           with the other head's QT rows
                                           zeroed; 1/8 scale folded into QT)
  PT = exp(ST) in bf16 (no max subtraction; scores are O(10), fp32-exp safe)
  causal masking by multiplying diagonal-band tiles with a bf16 0/1 mask
  OT_aug [65, q] += V_aug-chunk.T x PT    (bf16; V_aug = [V | ones]; row 64=l)
  OTn = OT * broadcast(1/l) + bv          (bf16; rows 64..127 kept zero)
  y[tok, :] += OTnz_h-chunk.T x Wo_h      (bf16, K=128 with zero rows)

Hardware rules honored here (all measured / verifier-enforced):
  - matmuls keep K=128: sustained K<128 matmuls run at the cold 1.2 GHz PE
    clock, doubling their cost.
  - f32r matmul operands must be produced by f32r-writing instructions
    (DMA of an f32r DRAM tensor, or ACT activations); DVE ops that write
    f32r run ~9x slow, so DVE only ever touches f32/bf16 tiles.
  - 1/l is broadcast across partitions with a DRAM-bounce DMA, not the PE.
"""

import numpy as np

import concourse.bass as bass
import concourse.mybir as mybir
import concourse.tile as tile
from concourse import bacc
from concourse.bass_utils import run_bass_kernel_spmd

D = 1024          # d_model
N = 2048          # sequence length
B = 2             # batch
H_TOT = 16        # total heads
HD = 64           # head dim
HPC = 4           # heads per core
NCORES = 8
SCALE = HD ** -0.5

F32 = mybir.dt.float32
F32R = mybir.dt.float32r
BF16 = mybir.dt.bfloat16

QTILE = 512       # q-tile width (free dim of score matmuls)
KCH = 128         # k-chunk (partition dim of score tiles)
NQT = N // QTILE  # 4
NKC = N // KCH    # 16
DCH = D // 128    # 8 d_model chunks
VROW = HD + 1     # V columns per head incl. ones column


def build_kernel():
    nc = bacc.Bacc("TRN2", target_bir_lowering=False, debug=False,
                   num_devices=NCORES)

    xT = nc.dram_tensor("xT", [D, N], F32R, kind="ExternalInput").ap()
    wq = nc.dram_tensor("wqT", [D, HPC * HD], F32R, kind="ExternalInput").ap()
    wk = nc.dram_tensor("wkT", [D, HPC * HD], F32R, kind="ExternalInput").ap()
    wv = nc.dram_tensor("wvT", [D, HPC * HD], F32R, kind="ExternalInput").ap()
    wo = nc.dram_tensor("woT", [HPC * HD, D], BF16, kind="ExternalInput").ap()
    bqz = nc.dram_tensor("bqz", [128, HPC], F32, kind="ExternalInput").ap()
    sclz = nc.dram_tensor("sclz", [128, HPC], F32, kind="ExternalInput").ap()
    bkd = nc.dram_tensor("bk", [HPC * HD], F32, kind="ExternalInput").ap()
    bvd = nc.dram_tensor("bv", [HPC * HD], F32, kind="ExternalInput").ap()
    maskd = nc.dram_tensor("mask", [128, 896], BF16, kind="ExternalInput").ap()
    y = nc.dram_tensor("y", [N, D], F32, kind="ExternalOutput").ap()

    Exp = mybir.ActivationFunctionType.Exp
    Identity = mybir.ActivationFunctionType.Identity

    with tile.TileContext(nc) as tc:
        from contextlib import ExitStack
        with ExitStack() as ctx:
            singles = ctx.enter_context(tc.tile_pool(name="singles", bufs=1))
            pt_pool = ctx.enter_context(tc.tile_pool(name="pt", bufs=4))
            r_pool = ctx.enter_context(tc.tile_pool(name="rp", bufs=2))
            yout = ctx.enter_context(tc.tile_pool(name="yout", bufs=3))
            dram = ctx.enter_context(
                tc.tile_pool(name="dram", bufs=2, space="DRAM"))
            ps_mm = ctx.enter_context(
                tc.tile_pool(name="psmm", bufs=2, space="PSUM"))
            ps_st = ctx.enter_context(
                tc.tile_pool(name="psst", bufs=2, space="PSUM"))
            ps_ot = ctx.enter_context(
                tc.tile_pool(name="psot", bufs=2, space="PSUM"))

            # --- resident inputs (DMA order = priority: first MMs need
            # wq/wk + x block 0) -------------------------------------------
            wq_sb = singles.tile([128, DCH, HPC * HD], F32R)
            nc.sync.dma_start(wq_sb[:], wq.rearrange("(o p) m -> p o m", p=128))
            wk_sb = singles.tile([128, DCH, HPC * HD], F32R)
            nc.sync.dma_start(wk_sb[:], wk.rearrange("(o p) m -> p o m", p=128))
            bqz_sb = singles.tile([128, HPC], F32)
            nc.sync.dma_start(bqz_sb[:], bqz)
            sclz_sb = singles.tile([128, HPC], F32)
            nc.sync.dma_start(sclz_sb[:], sclz)
            bk_sb = singles.tile([128, 2], F32)
            nc.sync.dma_start(bk_sb[:], bkd.rearrange("(o p) -> p o", p=128))

            # x, sliced [d-chunk, 512-token block] so compute can start as
            # soon as the first block's 2 MB lands.
            xk = [[singles.tile([128, QTILE], F32R, name=f"xk{k}_{b}")
                   for b in range(NQT)] for k in range(DCH)]
            for k in range(DCH):
                nc.sync.dma_start(
                    xk[k][0][:], xT[k * 128:(k + 1) * 128, 0:QTILE])

            wv_sb = singles.tile([128, DCH, HPC * HD], F32R)
            nc.sync.dma_start(wv_sb[:], wv.rearrange("(o p) m -> p o m", p=128))
            for b in range(1, NQT):
                for k in range(DCH):
                    nc.sync.dma_start(
                        xk[k][b][:],
                        xT[k * 128:(k + 1) * 128,
                           b * QTILE:(b + 1) * QTILE])

            woz = []
            for h in range(HPC):
                t = singles.tile([128, D], BF16, name=f"wo{h}")
                nc.sync.dma_start(t[:HD, :], wo[h * HD:(h + 1) * HD, :])
                nc.vector.memset(t[HD:, :], 0.0)
                woz.append(t)
            bv_sb = singles.tile([HD, HPC], F32)
            nc.sync.dma_start(bv_sb[:], bvd.rearrange("(h c) -> c h", c=HD))
            mask_sb = singles.tile([128, 896], BF16)
            nc.sync.dma_start(mask_sb[:], maskd)

            # PE warm-up: the first ~18us are DMA-bound with the PE idle,
            # which leaves the PE clock throttled to 1.2 GHz when real work
            # starts.  Issue dependency-free junk matmuls so the activity
            # monitor unthrottles before the first projection matmul.
            junk = singles.tile([128, 512], BF16)
            nc.vector.memset(junk[:], 0.0)
            for i in range(64):
                wps = ps_ot.tile([128, 512], F32, tag="ot", name="wps")
                nc.tensor.matmul(wps[:], lhsT=junk[:, :128], rhs=junk[:],
                                 start=True, stop=True)

            # QTz[h][qi]: [128, 512] with head h's 64 rows live at partition
            # offset (h%2)*64 and the other 64 rows zero, so score matmuls
            # contract over the full 128 partitions.
            QTz = [[singles.tile([128, QTILE], F32R, name=f"qtz{h}_{i}")
                    for i in range(NQT)] for h in range(HPC)]
            KT_sb = [singles.tile([128, 2, QTILE], F32R, name=f"kt{i}")
                     for i in range(NQT)]
            V_sb = [singles.tile([128, 4, HPC * VROW], BF16, name=f"v{i}")
                    for i in range(NQT)]
            OTnz = [singles.tile([128, N], BF16, name=f"otn{i}")
                    for i in range(HPC)]
            for h in range(HPC):
                nc.vector.memset(OTnz[h][HD:, :], 0.0)

            # --- stage A: projections --------------------------------------
            for blk in range(NQT):
                for m in range(2):
                    ps = ps_mm.tile([128, 512], F32, tag="mm")
                    for k in range(DCH):
                        nc.tensor.matmul(
                            ps[:],
                            lhsT=wq_sb[:, k, m * 128:(m + 1) * 128],
                            rhs=xk[k][blk][:],
                            start=(k == 0), stop=(k == DCH - 1))
                    for hh in range(2):
                        h = 2 * m + hh
                        # per-partition scale zeroes the other head's rows
                        nc.scalar.activation(
                            QTz[h][blk][:], ps[:], Identity,
                            bias=bqz_sb[:, h:h + 1],
                            scale=sclz_sb[:, h:h + 1])
                for m in range(2):
                    ps = ps_mm.tile([128, 512], F32, tag="mm")
                    for k in range(DCH):
                        nc.tensor.matmul(
                            ps[:],
                            lhsT=wk_sb[:, k, m * 128:(m + 1) * 128],
                            rhs=xk[k][blk][:],
                            start=(k == 0), stop=(k == DCH - 1))
                    nc.scalar.activation(
                        KT_sb[blk][:, m, :], ps[:], Identity,
                        bias=bk_sb[:, m:m + 1], scale=1.0)
                nc.vector.memset(V_sb[blk][:], 1.0)
                for tt in range(4):
                    ps = ps_mm.tile([128, 512], F32, tag="mm")
                    for k in range(DCH):
                        nc.tensor.matmul(
                            ps[:, :HPC * HD],
                            lhsT=xk[k][blk][:, tt * 128:(tt + 1) * 128],
                            rhs=wv_sb[:, k, :],
                            start=(k == 0), stop=(k == DCH - 1))
                    for h in range(HPC):
                        nc.vector.tensor_copy(
                            V_sb[blk][:, tt, h * VROW:h * VROW + HD],
                            ps[:, h * HD:(h + 1) * HD])

            # --- stages B+D interleaved: attention per q-block, then that
            # block's output projection (keeps the PE dense and lets the
            # y writeback overlap later blocks' attention) ------------------
            for qi in range(NQT):
                q0 = qi * QTILE
                for h in range(HPC):
                    mi = h // 2
                    nprs = 2 * (qi + 1)        # pairs of 128-k-chunks
                    pso = ps_ot.tile([VROW, 512], F32, tag="ot", name="pso")

                    pair_ps = [None] * nprs
                    pair_pt = [None] * nprs

                    def consume(pi):
                        pss, pt = pair_ps[pi], pair_pt[pi]
                        nc.scalar.activation(pt[:], pss[:], Exp)
                        for j in range(2):
                            ki = 2 * pi + j
                            r = ki * KCH - q0
                            if r >= 0:
                                nc.vector.tensor_mul(
                                    pt[:, j, :], pt[:, j, :],
                                    mask_sb[:, 384 - r:384 - r + 512])
                        for j in range(2):
                            ki = 2 * pi + j
                            nc.tensor.matmul(
                                pso[:],
                                lhsT=V_sb[ki // 4][:, ki % 4,
                                                   h * VROW:(h + 1) * VROW],
                                rhs=pt[:, j, :],
                                start=(ki == 0), stop=(ki == 4 * (qi + 1) - 1))

                    for pi in range(nprs):
                        pss = ps_st.tile([128, 2, 512], F32, tag="st",
                                         name="pss")
                        pair_ps[pi] = pss
                        pair_pt[pi] = pt_pool.tile([128, 2, 512], BF16,
                                                   tag="pt", name="pt")
                        for j in range(2):
                            ki = 2 * pi + j
                            nc.tensor.matmul(
                                pss[:, j, :],
                                lhsT=KT_sb[ki // 4][:, mi,
                                                    (ki % 4) * 128:
                                                    (ki % 4) * 128 + 128],
                                rhs=QTz[h][qi][:],
                                start=True, stop=True)
                        if pi > 0:
                            consume(pi - 1)
                    consume(nprs - 1)

                    # Copy OT_aug off PSUM immediately so the psum slot
                    # frees; then normalize off-PSUM: broadcast l across 64
                    # partitions via a DRAM bounce, reciprocal in parallel.
                    otu = r_pool.tile([VROW, 512], F32, tag="otu", name="otu")
                    nc.vector.tensor_copy(otu[:], pso[:])
                    sc = dram.tile([1, 512], F32, tag="sc", name="sc")
                    nc.sync.dma_start(sc[:], otu[HD:HD + 1, :])
                    rb = r_pool.tile([HD, 512], F32, tag="rb", name="rb")
                    row = sc[0, :]
                    bcast = bass.AP(tensor=row.tensor, offset=row.offset,
                                    ap=[[0, HD]] + list(row.ap))
                    nc.sync.dma_start(rb[:], bcast)
                    nc.vector.reciprocal_approx_fast(out=rb[:], in_=rb[:])
                    nc.vector.tensor_mul(
                        OTnz[h][:HD, q0:q0 + QTILE], otu[:HD, :], rb[:])
                    nc.vector.tensor_scalar_add(
                        OTnz[h][:HD, q0:q0 + QTILE],
                        OTnz[h][:HD, q0:q0 + QTILE],
                        bv_sb[:, h:h + 1])

                # output projection for this q-block's 4 token chunks
                for tt in range(4):
                    t0 = q0 + tt * 128
                    for half in range(2):
                        ps = ps_mm.tile([128, 512], F32, tag="mm", name="psy")
                        for h in range(HPC):
                            nc.tensor.matmul(
                                ps[:],
                                lhsT=OTnz[h][:, t0:t0 + 128],
                                rhs=woz[h][:, half * 512:half * 512 + 512],
                                start=(h == 0), stop=(h == HPC - 1))
                        yt = yout.tile([128, 512], F32, tag="y", name="yt")
                        nc.vector.tensor_copy(yt[:], ps[:])
                        nc.sync.dma_start(
                            y[t0:t0 + 128, half * 512:half * 512 + 512],
                            yt[:])

    nc.compile()
    return nc


def make_in_maps(x, Wq, bq, Wkv, bkv, Wo, bo):
    import ml_dtypes

    x = np.asarray(x, np.float32)
    Wq = np.asarray(Wq, np.float32)
    bq = np.asarray(bq, np.float32)
    Wkv = np.asarray(Wkv, np.float32)
    bkv = np.asarray(bkv, np.float32)
    Wo = np.asarray(Wo, np.float32)

    Wk, Wv = Wkv[:D], Wkv[D:]
    bk, bv = bkv[:D], bkv[D:]

    # mask[kk, u] = 1 iff u >= kk + 384 ; slice [384-r : 896-r] gives the
    # keep-mask (q >= k + r) for a diagonal chunk with offset r.
    u = np.arange(896)[None, :]
    kk = np.arange(128)[:, None]
    mask = (u >= kk + 384).astype(ml_dtypes.bfloat16)

    in_maps = []
    for c in range(NCORES):
        b = c // (NCORES // B)
        hs = HPC * (c % (NCORES // B))
        rows = slice(hs * HD, hs * HD + HPC * HD)
        bq_c = bq[rows] * SCALE
        # bqz/sclz: per-head column, live on that head's 64 partitions only
        bqz = np.zeros((128, HPC), np.float32)
        sclz = np.zeros((128, HPC), np.float32)
        for h in range(HPC):
            po = (h % 2) * 64
            m = h // 2
            bqz[po:po + 64, h] = bq_c[m * 128 + po:m * 128 + po + 64]
            sclz[po:po + 64, h] = SCALE
        in_maps.append({
            "xT": np.ascontiguousarray(x[b].T),
            "wqT": np.ascontiguousarray(Wq[rows].T),
            "wkT": np.ascontiguousarray(Wk[rows].T),
            "wvT": np.ascontiguousarray(Wv[rows].T),
            "woT": np.ascontiguousarray(Wo[:, rows].T).astype(
                ml_dtypes.bfloat16),
            "bqz": bqz,
            "sclz": sclz,
            "bk": np.ascontiguousarray(bk[rows]),
            "bv": np.ascontiguousarray(bv[rows]),
            "mask": mask,
        })
    return in_maps


_NC_CACHE = None


def _get_nc():
    global _NC_CACHE
    if _NC_CACHE is None:
        _NC_CACHE = build_kernel()
    return _NC_CACHE


def kernel(x, Wq, bq, Wkv, bkv, Wo, bo, _trace=False, _trace_kwargs=None):
    nc = _get_nc()
    in_maps = make_in_maps(x, Wq, bq, Wkv, bkv, Wo, bo)
    kwargs = {}
    if _trace:
        kwargs = dict(trace=True, trace_cores=list(range(NCORES)),
                      **(_trace_kwargs or {}))
    res = run_bass_kernel_spmd(nc, in_maps, core_ids=list(range(NCORES)),
                               **kwargs)
    out = np.zeros((B, N, D), np.float32)
    for c, r in enumerate(res.results):
        out[c // (NCORES // B)] += r["y"]
    out += np.asarray(bo, np.float32)[None, None, :]
    if _trace:
        kernel.last_results = res
    return out
